# revision 7
# baseline (speedup 1.0000x reference)
"""Deep Richardson-Lucy deconvolution on 8 Trainium2 NeuronCores.

Strategy (per core, data-parallel batch shard of 512 rows):
- Transposed fp16 SBUF layout: [l on partitions (64 blocks of 128), batch
  on the free dim (512)]. Host does the transpose/cast staging.
- conv2 (and conv1 in the last 2 iterations): banded-Toeplitz matmuls per
  128-l block: center [128,128] fp16 matmul + two 32-row halo matmuls
  reading the neighbor tiles' partition subranges directly; the halos land
  in disjoint PE quadrants and stream concurrently.
- conv1 in iterations 1..7 runs as ONE fp8 DoubleRow matmul per block:
  the [128,2,128] e4m3 weight + [128,2,512] rhs (two adjacent half-shifted
  fp8 s-tiles) contract over 256 rows, covering the whole 158-wide conv
  band in a single N=512 stream at 2x fp8 rate. Weights use two
  complementary quantizations (A/B, alternated per iteration) whose exact
  normalization folds into the ACT reciprocal scale. fp8 noise from early
  iterations is damped by the RL fixed-point dynamics (validated vs fp64).
- s stays fp16 master; a half-shifted e4m3 shadow (s8s) is rebuilt each
  iteration by a cheap cast (s16->fp8 ring) + bulk same-size SBUF->SBUF
  DMAs that apply the 64-partition shift.
- Elementwise: ACT does recip + 3/4 of the conv2-psum evacuation
  (+EPS copies); DVE does ratio muls, fp8 casts, the remaining STT
  updates and the evac-path muls. gpsimd only does startup memsets (its
  tensor ops are ~15x slower and stall DVE via the shared SBUF port).
- A dummy-matmul warmup burst at t=0 gets the PE HAM clock gate to 2.4GHz
  by ~4us instead of ~115us.
"""
import hashlib
import numpy as np

EPS = 1e-6
P = 128
KTAPS = 31
PAD = 15
B_FULL, L = 4096, 8192
N_CORES = 8
BC = B_FULL // N_CORES          # 512 batch rows per core
NT = L // P                     # 64 l-blocks
NITER = 10
NQ = NT // 4                    # 16 quads
DR_LAST = 8                     # conv1 DoubleRow for iters 1..DR_LAST-1

_cache = {}


# ---------------- host-side weight/constant packing ----------------

def _e4m3_q(x):
    import ml_dtypes
    return np.clip(np.asarray(x, dtype=np.float32), -240, 240).astype(
        ml_dtypes.float8_e4m3).astype(np.float64)


def _quant_taps(target, n_c=2000):
    """Best e4m3 quantization of 31 taps under a free post-scale:
    minimize normalized shape error."""
    target = np.asarray(target, dtype=np.float64)
    tot = target.sum()
    best = None
    for c in np.geomspace(100, 3000, n_c):
        q = _e4m3_q(target * c)
        h = q / q.sum() * tot
        d = np.abs(h - target)
        e = d.sum() + 5 * d.max()
        if best is None or e < best[0]:
            best = (e, q)
    q = best[1]
    return q, float(tot / q.sum())


def _quant_pair(psf):
    """A/B complementary quantizations: avg of effective kernels ~ psf."""
    psf = np.asarray(psf, dtype=np.float64)
    qa, pa = _quant_taps(psf)
    ha = qa * pa
    target = 2 * psf - ha
    tot = target.sum()
    best = None
    for c in np.geomspace(100, 3000, 2000):
        q = _e4m3_q(target * c)
        h = q / q.sum() * tot
        d = np.abs((ha + h) / 2 - psf)
        e = d.sum() + 5 * d.max()
        if best is None or e < best[0]:
            best = (e, q)
    qb = best[1]
    pb = float(tot / qb.sum())
    return (qa, pa), (qb, pb)


def _build_toeplitz(psf):
    Wc = np.zeros((P, P), dtype=np.float64)
    j = np.arange(P)[:, None]
    i = np.arange(P)[None, :]
    k = j - i + PAD
    m = (k >= 0) & (k < KTAPS)
    Wc[m] = psf[k[m]]
    WL = np.zeros((32, 32), dtype=np.float64)   # rhs = prev block parts [96,128)
    jj = np.arange(32)[:, None]
    ii = np.arange(32)[None, :]
    k = (96 + jj - 128) - ii + PAD
    m = (k >= 0) & (k < KTAPS)
    WL[m] = psf[k[m]]
    WR = np.zeros((32, 32), dtype=np.float64)   # rhs = next block parts [0,32)
    k = (jj + 128) - (96 + ii) + PAD
    m = (k >= 0) & (k < KTAPS)
    WR[m] = psf[k[m]]
    return Wc, WL, WR


def _wpack(psf):
    """[P, 256] fp16: cols 0:128 = center Toeplitz; w[0:32,224:256]=WR
    (-> out 96:128 from next block rows 0:32); w[96:128,128:160]=WL
    (-> out 0:32 from prev block rows 96:128)."""
    Wc, WL, WR = _build_toeplitz(psf)
    w = np.zeros((P, 256), dtype=np.float16)
    w[:, 0:128] = Wc
    w[0:32, 128 + 96:128 + 128] = WR
    w[96:128, 128 + 0:128 + 32] = WL
    return w


def _wdr_pack(taps):
    """DoubleRow conv1 weight [P, 2, 128] fp8e4 (as raw fp32 values for
    host conversion). Input: half-shifted s8s tiles (u, u+1); tile u
    partition p holds l = 128u - 64 + p. Output: aligned block u, col j
    holds l_out = 128u + j. Tap index tau = (l_in - l_out) + 15.
    slot o=0 (tile u):   l_in = 128u - 64 + p  -> tau = p - j - 49
    slot o=1 (tile u+1): l_in = 128u + 64 + p  -> tau = p - j + 79
    """
    w = np.zeros((P, 2, P), dtype=np.float64)
    pidx = np.arange(P)[:, None]
    jidx = np.arange(P)[None, :]
    t0 = pidx - jidx - 49
    m0 = (t0 >= 0) & (t0 < KTAPS)
    w[:, 0, :][m0] = taps[t0[m0]]
    t1 = pidx - jidx + 79
    m1 = (t1 >= 0) & (t1 < KTAPS)
    w[:, 1, :][m1] = taps[t1[m1]]
    return w


def _r0pack(psf64):
    """r0[p, t] = 1 / (conv1d(0.5*ones, psf)[128t+p] + EPS)."""
    ones = np.full((1, L), 0.5, dtype=np.float64)
    xp = np.pad(ones, ((0, 0), (PAD, PAD)))
    sc = np.zeros((1, L), dtype=np.float64)
    for k in range(KTAPS):
        sc += xp[:, k:k + L] * psf64[k]
    r = 1.0 / (sc[0] + EPS)
    return r.reshape(NT, P).T.astype(np.float32)


# ---------------- device kernel ----------------

def _build(psf64, alpha64):
    import concourse.bass as bass
    import concourse.tile as tile
    from concourse import mybir
    import bass_rust

    F32 = mybir.dt.float32
    F16 = mybir.dt.float16
    F8 = mybir.dt.float8e4

    class SafeTC(tile.TileContext):
        # this walrus build rejects >1 sync wait per CTRL-class instruction
        def _drain_and_barrier(self, tick_clock, wait_clock):
            gc = tick_clock.global_clock
            for i in range(len(gc)):
                if gc[i] > 0:
                    di = self.nc.sync.drain()
                    pc = bass_rust.VectorClock()
                    pc.require_at_least(i, gc[i])
                    wait_clock.add_sem_waits(di.ins, bass_rust.ScopedClock({None: pc}))
            self.nc.all_engine_barrier()
            popped = self.nc._tile_sem_poison_stack.pop()
            assert popped is self._sem_poison
            self.nc.clear_and_free_semaphores(list(self.sems.allocated().values()))
            self.nc.all_engine_barrier()

    def split_multi_waits(nc, max_waits=1):
        n_fixed = 0
        uid = [0]
        for f in nc.m.functions:
            for bb in f.blocks:
                out = []
                changed = False
                for inst in bb.instructions:
                    si = inst.sync_info
                    if si is not None:
                        sems = [w for w in si.on_wait
                                if str(getattr(w, "sync_type", "")) == "semaphore"]
                        other = [w for w in si.on_wait if w not in sems]
                        if len(sems) > max_waits:
                            keep = sems[-max_waits:]
                            for w in sems[:-max_waits]:
                                nop = mybir.InstNoOp(
                                    name=f"waitsplit_{uid[0]}", ins=[], outs=[])
                                uid[0] += 1
                                nop.engine = inst.engine
                                nop.sync_info = mybir.SyncInfo(
                                    on_wait=[w], on_update=[])
                                out.append(nop)
                            inst.sync_info = mybir.SyncInfo(
                                on_wait=other + keep,
                                on_update=list(si.on_update))
                            n_fixed += 1
                            changed = True
                    out.append(inst)
                if changed:
                    try:
                        bb.instructions = out
                    except Exception:
                        bb.instructions.clear()
                        bb.instructions.extend(out)
        return n_fixed

    def act_raw(nc, out, in_, func, bias=0.0, scale=1.0):
        eng = nc.scalar
        ins = [eng.lower_ap(in_),
               mybir.ImmediateValue(dtype=F32, value=float(bias)),
               mybir.ImmediateValue(dtype=F32, value=float(scale)),
               mybir.ImmediateValue(dtype=F32, value=0.0)]
        return eng.add_instruction(mybir.InstActivation(
            name=nc.get_next_instruction_name(), func=func, ins=ins,
            outs=[eng.lower_ap(out)]))

    alpha_is_one = bool(np.all(alpha64 == 1.0))

    nc = bass.Bass("TRN2", target_bir_lowering=False, debug=False,
                   num_devices=N_CORES)
    mT_in = nc.dram_tensor("mT", [P, NT, BC], F16, kind="ExternalInput")
    w1_in = nc.dram_tensor("w1", [P, 256], F16, kind="ExternalInput")
    w2_in = nc.dram_tensor("w2", [P, 256], F16, kind="ExternalInput")
    w1dra_in = nc.dram_tensor("w1dra", [P, 2, P], F8, kind="ExternalInput")
    w1drb_in = nc.dram_tensor("w1drb", [P, 2, P], F8, kind="ExternalInput")
    r0_in = nc.dram_tensor("r0", [P, NT], F32, kind="ExternalInput")
    y_out = nc.dram_tensor("y", [P, NT, BC], F16, kind="ExternalOutput")

    Rec = mybir.ActivationFunctionType.Reciprocal
    Ln = mybir.ActivationFunctionType.Ln
    Exp = mybir.ActivationFunctionType.Exp
    Copy = mybir.ActivationFunctionType.Copy
    DR = mybir.MatmulPerfMode.DoubleRow

    def conv_cph(psum, w, src, t):
        """fp16 center + two 32-row halo matmuls reading src tiles'
        partition subranges directly; halos go to disjoint PE quadrants.
        src(t) must return the [P, BC] fp16 tile for block t."""
        last = "R" if t < NT - 1 else "L"
        nc.tensor.matmul(psum, w[:, 0:128], src(t),
                         start=True, stop=False)
        if t > 0:
            nc.tensor.matmul(psum[0:32, :], w[96:128, 128:160],
                             src(t - 1)[96:128, :], start=False,
                             stop=(last == "L"), tile_position=(96, 0))
        if t < NT - 1:
            nc.tensor.matmul(psum[96:128, :], w[0:32, 224:256],
                             src(t + 1)[0:32, :], start=False,
                             stop=(last == "R"), tile_position=(0, 96))

    with SafeTC(nc) as tc:
        with tc.tile_pool(name="wpool", bufs=1) as wpool, \
             tc.tile_pool(name="mpool", bufs=1) as mpool, \
             tc.tile_pool(name="spool", bufs=1) as spool:
            w1 = wpool.tile([P, 256], F16)
            nc.sync.dma_start(w1[:], w1_in[:])
            w2 = wpool.tile([P, 256], F16)
            nc.sync.dma_start(w2[:], w2_in[:])
            w1dr_a = wpool.tile([P, 2, P], F8)
            w1dr_b = wpool.tile([P, 2, P], F8)
            w1dr = [w1dr_a, w1dr_b]
            nc.sync.dma_start(w1dr[0][:], w1dra_in[:])
            nc.sync.dma_start(w1dr[1][:], w1drb_in[:])
            r0 = wpool.tile([P, NT], F32)
            nc.sync.dma_start(r0[:], r0_in[:])
            r0f = r0

            # PE warmup fodder: small fp16 tile, memset -> finite values.
            wu = wpool.tile([P, 512], F16)
            nc.vector.memset(wu[:], 0.125)

            mT = mpool.tile([P, NT, BC], F16)
            # chunked loads across 8 DMA queues
            for q in range(8):
                nc.sync.dma_start(mT[:, q * 8:(q + 1) * 8, :],
                                  mT_in[:, q * 8:(q + 1) * 8, :])
            s = spool.tile([P, NT, BC], F16)
            # half-shifted fp8 shadow of s: tile u part p holds l=128u-64+p
            s8s = spool.tile([P, NT + 1, BC], F8)
            nc.gpsimd.memset(s8s[0:64, 0, :], 0.0)
            nc.gpsimd.memset(s8s[64:128, NT, :], 0.0)

            with tc.tile_pool(name="ratio", bufs=4) as rpool, \
                 tc.tile_pool(name="rtile", bufs=3) as rtp, \
                 tc.tile_pool(name="cptile", bufs=3) as cpp, \
                 tc.tile_pool(name="cast8", bufs=2) as cpool, \
                 tc.tile_pool(name="psum", bufs=4, space="PSUM") as pp:

                # ---- PE warmup: ~24 back-to-back matmuls (~5us) so the
                # HAM clock-gate opens to 2.4GHz before real work starts.
                wps = pp.tile([P, 2, BC], F32, tag="ps")
                for i in range(24):
                    nc.tensor.matmul(wps[:, i % 2, :], wu[:, 0:128],
                                     wu[:], start=True, stop=True)

                for it in range(NITER):
                    dr_iter = 1 <= it < DR_LAST
                    need_cast = it < DR_LAST - 1   # s8s consumed by iters 1..DR_LAST-1
                    wpar = it % 2
                    ratio_quads = [None] * NQ
                    rt_quads = [None] * NQ

                    def rat(t):
                        return ratio_quads[t // 4][:, t % 4, :]

                    def s16(t):
                        return s[:, t, :]

                    def _conv1_recip_pair(j):
                        u = 2 * j
                        q4 = j // 2
                        if j % 2 == 0:
                            rtq = rtp.tile([P, 4, BC], F16, tag="rtq")
                            rt_quads[q4] = rtq
                        ps = pp.tile([P, 2, BC], F32, tag="ps")
                        if dr_iter:
                            nc.tensor.matmul(ps[:, 0, :], w1dr[wpar][:],
                                             s8s[:, u:u + 2, :],
                                             start=True, stop=True,
                                             perf_mode=DR)
                            nc.tensor.matmul(ps[:, 1, :], w1dr[wpar][:],
                                             s8s[:, u + 1:u + 3, :],
                                             start=True, stop=True,
                                             perf_mode=DR)
                            rscale = _DR_POST[wpar]
                        else:
                            conv_cph(ps[:, 0, :], w1, s16, u)
                            conv_cph(ps[:, 1, :], w1, s16, u + 1)
                            rscale = 1.0
                        half = (j % 2) * 2
                        act_raw(nc, rt_quads[q4][:, half:half + 2, :],
                                ps[:], Rec, bias=EPS, scale=rscale)

                    def _ratio_quad(q4):
                        u = 4 * q4
                        ra = rpool.tile([P, 4, BC], F16, tag="ra")
                        if it == 0:
                            for k in range(4):
                                nc.scalar.activation(
                                    ra[:, k, :], mT[:, u + k, :], Copy,
                                    bias=0.0,
                                    scale=r0f[:, u + k:u + k + 1])
                        else:
                            nc.vector.tensor_mul(ra[:], mT[:, u:u + 4, :],
                                                 rt_quads[q4][:])
                        ratio_quads[q4] = ra

                    def _cast_quad(q4):
                        """s16 quad -> fp8 ring tile -> shifted s8s tiles
                        via 64-partition SBUF->SBUF DMA copies."""
                        u = 4 * q4
                        c8 = cpool.tile([P, 4, BC], F8, tag="c8")
                        nc.vector.tensor_copy(c8[:], s[:, u:u + 4, :])
                        # upper halves: s8s[64:128, u+k] <- c8 tile k parts 0:64
                        nc.sync.dma_start(s8s[64:128, u:u + 4, :],
                                          c8[0:64, :, :])
                        # lower halves: s8s[0:64, u+k+1] <- c8 tile k parts 64:128
                        nc.sync.dma_start(s8s[0:64, u + 1:u + 5, :],
                                          c8[64:128, :, :])
                        return c8

                    def _conv2_update_pair(j):
                        u = 2 * j
                        ps = pp.tile([P, 2, BC], F32, tag="ps")
                        conv_cph(ps[:, 0, :], w2, rat, u)
                        conv_cph(ps[:, 1, :], w2, rat, u + 1)
                        if alpha_is_one and it == 0:
                            nc.vector.tensor_scalar(
                                out=s[:, u:u + 2, :], in0=ps[:],
                                scalar1=EPS, scalar2=0.5,
                                op0=mybir.AluOpType.add,
                                op1=mybir.AluOpType.mult)
                            # HAM fodder: it0 is elementwise-bound, so the
                            # PE has idle slots; keep the clock gate open.
                            nc.tensor.matmul(ps[:, 0, :], wu[:, 0:128],
                                             wu[:], start=True, stop=True)
                            nc.tensor.matmul(ps[:, 1, :], wu[:, 0:128],
                                             wu[:], start=True, stop=True)
                        elif alpha_is_one:
                            if j % 3 == 0:
                                # DVE fused: s = (psum + EPS) * s
                                nc.vector.scalar_tensor_tensor(
                                    out=s[:, u:u + 2, :], in0=ps[:],
                                    scalar=EPS, in1=s[:, u:u + 2, :],
                                    op0=mybir.AluOpType.add,
                                    op1=mybir.AluOpType.mult)
                            else:
                                # ACT evacuates PSUM (+EPS), Pool fp16 mul
                                cp = cpp.tile([P, 2, BC], F16, tag="cp")
                                act_raw(nc, cp[:], ps[:], Copy, bias=EPS)
                                nc.vector.tensor_mul(s[:, u:u + 2, :],
                                                     s[:, u:u + 2, :], cp[:])
                        else:
                            lg = cpp.tile([P, 2, BC], F32, tag="lg")
                            act_raw(nc, lg[:], ps[:], Ln, bias=EPS)
                            cp = cpp.tile([P, 2, BC], F16, tag="cp")
                            act_raw(nc, cp[:], lg[:], Exp,
                                    scale=float(alpha64[it]))
                            nc.vector.tensor_mul(s[:, u:u + 2, :],
                                                 s[:, u:u + 2, :], cp[:])
                        if need_cast and j % 2 == 1:
                            _cast_quad(j // 2)
                        if it == NITER - 1 and (j + 1) % 8 == 0:
                            q = j // 8
                            nc.sync.dma_start(
                                y_out[:, q * 16:(q + 1) * 16, :],
                                s[:, q * 16:(q + 1) * 16, :])

                    def _produce_quad(q4):
                        if it > 0:
                            _conv1_recip_pair(2 * q4)
                            _conv1_recip_pair(2 * q4 + 1)
                        _ratio_quad(q4)

                    _produce_quad(0)
                    _produce_quad(1)
                    for qq in range(NQ):
                        if qq + 2 < NQ:
                            _produce_quad(qq + 2)
                        _conv2_update_pair(2 * qq)
                        _conv2_update_pair(2 * qq + 1)

    split_multi_waits(nc)
    return nc


_DR_POST = [1.0, 1.0]   # filled by _prepare (A/B post-scales)


def _prepare(psf64):
    """Host-side packs that depend on psf."""
    global _DR_POST
    (qa, pa), (qb, pb) = _quant_pair(psf64)
    _DR_POST = [pa, pb]
    w1 = _wpack(psf64)
    w2 = _wpack(psf64[::-1])
    import ml_dtypes
    w1dra = np.clip(_wdr_pack(qa), -240, 240).astype(np.float32).astype(
        ml_dtypes.float8_e4m3)
    w1drb = np.clip(_wdr_pack(qb), -240, 240).astype(np.float32).astype(
        ml_dtypes.float8_e4m3)
    r0 = _r0pack(psf64)
    return w1, w2, w1dra, w1drb, r0


def _make_in_maps(m, psf, alpha):
    m = np.asarray(m)
    psf64 = np.asarray(psf, dtype=np.float64)
    w1, w2, w1dra, w1drb, r0 = _prepare(psf64)
    in_maps = []
    for c in range(N_CORES):
        mc = m[c * BC:(c + 1) * BC].astype(np.float16)      # [BC, L]
        mT = np.ascontiguousarray(
            mc.reshape(BC, NT, P).transpose(2, 1, 0))        # [P, NT, BC]
        in_maps.append({"mT": mT, "w1": w1, "w2": w2,
                        "w1dra": w1dra, "w1drb": w1drb, "r0": r0})
    return in_maps


def kernel(m, psf, alpha):
    m = np.asarray(m)
    psf64 = np.asarray(psf, dtype=np.float64)
    alpha64 = np.asarray(alpha, dtype=np.float64)
    key = hashlib.sha256(
        psf64.tobytes() + alpha64.tobytes() + str(m.shape).encode()).hexdigest()
    if key not in _cache:
        _prepare(psf64)          # sets _DR_POST before _build captures it
        _cache[key] = _build(psf64, alpha64)
    nc = _cache[key]

    from concourse.bass_utils import run_bass_kernel_spmd
    in_maps = _make_in_maps(m, psf, alpha)
    res = run_bass_kernel_spmd(nc, in_maps, core_ids=list(range(N_CORES)))
    outs = []
    for c in range(N_CORES):
        yT = res.results[c]["y"]                             # [P, NT, BC] fp16
        outs.append(np.asarray(yT).transpose(2, 1, 0).reshape(BC, L))
    return np.concatenate(outs, axis=0).astype(np.float32)
